# revision 1
# baseline (speedup 1.0000x reference)
"""GAT layer (nn_GATLayer_28106265985525) on 8 Trainium2 NeuronCores.

Batch-parallel: core b computes graph b (bs=8). Per core:
  nodes   = x @ W.T + b                         [N, F]   (fp32 on PE)
  f_src   = nodes @ a1,  f_dst = nodes @ a2     [N]
  s[i,j]  = lrelu(f_src[i] + f_dst[j] + a_b)    masked by adj (log-space -150)
  attn    = softmax_j(s);  out = attn @ nodes   [N, F]

Main loop per 128-row block: SWDGE DMA streams adj with an inline
int32->f16 cast; DVE fuses mask-affine + f_src bias + f_dst add into one
affine_then_add, then lrelu = max(x, 0.2x) via scalar_tensor_tensor;
PE transposes the 16 score tiles into PSUM (fp16, 1 cyc/row); ACT does a
single Exp pass that simultaneously evacuates PSUM->SBUF; PE accumulates
out += E_T.T @ nodesE where nodesE carries a ones column so the softmax
denominator Z falls out of the same matmuls; DVE reciprocal + scale; DMA out.
The -150 log-space mask makes masked scores ~0.2s-30, whose exp underflows
to exactly 0 in f16 -- equivalent to the reference's -1e32 masking.
"""

import numpy as np
from contextlib import ExitStack

N = 2048
FIN = 256
F = 128
BS = 8
TB = N // 128  # 16 row blocks
NEG_BIG = 150.0
ALPHA = 0.2

_cache = {}


def _build(reps=1):
    import concourse.bass as bass
    import concourse.tile as tile
    from concourse import mybir, bacc

    f32, f16, i32 = mybir.dt.float32, mybir.dt.float16, mybir.dt.int32
    A = mybir.AluOpType
    AF = mybir.ActivationFunctionType

    nc = bacc.Bacc("TRN2", target_bir_lowering=False, debug=False)
    xt_d = nc.declare_dram_parameter("xT", [FIN, N], f32, isOutput=False)
    adj_d = nc.declare_dram_parameter("adj", [N, N], i32, isOutput=False)
    wt_d = nc.declare_dram_parameter("WwT", [FIN, F], f32, isOutput=False)
    wb_d = nc.declare_dram_parameter("Wb", [F, 1], f32, isOutput=False)
    a12_d = nc.declare_dram_parameter("a12", [F, 2], f32, isOutput=False)
    ab2_d = nc.declare_dram_parameter("ab2", [2, 1], f32, isOutput=False)
    idf16_d = nc.declare_dram_parameter("idf16", [128, 128], f16, isOutput=False)
    out_d = nc.declare_dram_parameter("out", [N, F], f32, isOutput=True)

    with tile.TileContext(nc) as tc, ExitStack() as ctx:
        consts = ctx.enter_context(tc.tile_pool(name="consts", bufs=1))
        setup = ctx.enter_context(tc.tile_pool(name="setup", bufs=2))
        adjp = ctx.enter_context(tc.tile_pool(name="adjp", bufs=8))
        work = ctx.enter_context(tc.tile_pool(name="work", bufs=3))
        outp = ctx.enter_context(tc.tile_pool(name="outp", bufs=2))
        ps_set = ctx.enter_context(tc.tile_pool(name="ps_set", bufs=2, space="PSUM"))
        ps_smT = ctx.enter_context(tc.tile_pool(name="ps_smT", bufs=2, space="PSUM"))
        ps_out = ctx.enter_context(tc.tile_pool(name="ps_out", bufs=2, space="PSUM"))

        idf16 = consts.tile([128, 128], f16)
        nc.sync.dma_start(idf16[:], idf16_d[:, :])

        # ---- setup: node features (xT / WwT arrive pre-transposed) ----
        xt_sb = consts.tile([128, 2 * N], f32)
        nc.gpsimd.dma_start(
            xt_sb[:].rearrange("p (c n) -> p c n", c=2),
            xt_d[:, :].rearrange("(c p) n -> p c n", p=128),
        )
        wt_sb = consts.tile([128, FIN], f32)
        nc.gpsimd.dma_start(
            wt_sb[:].rearrange("p (c o) -> p c o", c=2),
            wt_d[:, :].rearrange("(c p) o -> p c o", p=128),
        )
        wb_col = consts.tile([128, 1], f32)
        nc.gpsimd.dma_start(wb_col[:], wb_d[:, :])
        a12 = consts.tile([128, 2], f32)
        nc.gpsimd.dma_start(a12[:], a12_d[:, :])
        ab2_sb = consts.tile([2, 1], f32)
        nc.gpsimd.dma_start(ab2_sb[:], ab2_d[:, :])

        # nodes_T = W @ x^T + b : nT_sb [o(128), n(2048)] fp32
        nT_sb = consts.tile([128, N], f32)
        for nch in range(4):
            nT_ps = ps_set.tile([128, 512], f32, tag="s")
            for c in range(2):
                nc.tensor.matmul(
                    nT_ps[:],
                    wt_sb[:, c * 128:(c + 1) * 128],
                    xt_sb[:, c * N + nch * 512: c * N + nch * 512 + 512],
                    start=(c == 0),
                    stop=(c == 1),
                )
            nc.scalar.activation(
                nT_sb[:, nch * 512:(nch + 1) * 512], nT_ps[:],
                AF.Identity, bias=wb_col[:], scale=1.0,
            )

        # f rows: [2, 2048], row0 = f_src, row1 = f_dst
        f_sb = consts.tile([2, N], f32)
        for nch in range(4):
            f_ps = ps_set.tile([2, 512], f32, tag="s")
            nc.tensor.matmul(
                f_ps[:], a12[:],
                nT_sb[:, nch * 512:(nch + 1) * 512],
                start=True, stop=True,
            )
            nc.scalar.activation(
                f_sb[:, nch * 512:(nch + 1) * 512], f_ps[:],
                AF.Identity, bias=ab2_sb[:], scale=1.0,
            )

        # fs_cols [128, 16]: column-major strided reload of f_src, minus NEG_BIG
        fs_dram = nc.dram_tensor("fs_scratch", [1, N], f32)
        nc.gpsimd.dma_start(fs_dram[:, :], f_sb[0:1, :])
        fs_raw = setup.tile([128, TB], f32, tag="fsr")
        nc.gpsimd.dma_start(
            fs_raw[:], fs_dram[0, :].rearrange("(t p) -> p t", p=128)
        )
        fs_cols = consts.tile([128, TB], f32)
        nc.vector.tensor_scalar(fs_cols[:], fs_raw[:], -NEG_BIG, None, A.add)

        # fd_bcast f16 [128, 2048]: A2B[o, m] = a2[o]; fdb = A2B.T @ nodes_T
        a2b = consts.tile([128, 128], f32)
        nc.vector.memset(a2b[:], 1.0)
        nc.vector.tensor_scalar(a2b[:], a2b[:], a12[:, 1:2], None, A.mult)
        fdb_sb = consts.tile([128, N], f16)
        for nch in range(4):
            fb_ps = ps_set.tile([128, 512], f32, tag="s")
            nc.tensor.matmul(
                fb_ps[:], a2b[:],
                nT_sb[:, nch * 512:(nch + 1) * 512],
                start=True, stop=True,
            )
            nc.scalar.copy(fdb_sb[:, nch * 512:(nch + 1) * 512], fb_ps[:])

        # nodesE f16 [128, 16*130]: nodes tiles + ones col + zero pad
        nT16 = setup.tile([128, N], f16, tag="nT16")
        nc.vector.tensor_copy(nT16[:], nT_sb[:])
        nE_sb = consts.tile([128, TB * 130], f16)
        nE_v = nE_sb[:].rearrange("p (t e) -> p t e", e=130)
        for g in range(4):
            nE_ps = ps_set.tile([128, 512], f16, tag="s")
            for k in range(4):
                t = g * 4 + k
                nc.tensor.transpose(
                    nE_ps[:, k * 128:(k + 1) * 128],
                    nT16[:, t * 128:(t + 1) * 128],
                    idf16[:],
                )
            nc.scalar.copy(
                nE_v[:, g * 4:(g + 1) * 4, 0:128],
                nE_ps[:].rearrange("p (k e) -> p k e", e=128),
            )
        nc.vector.memset(nE_v[:, :, 128:129], 1.0)
        nc.vector.memset(nE_v[:, :, 129:130], 0.0)

        # ---- main loop over row blocks ----
        for t in [t for _ in range(reps) for t in range(TB)]:
            adj_t = adjp.tile([128, N], f16, tag="adj")
            nc.gpsimd.dma_start(adj_t[:], adj_d[t * 128:(t + 1) * 128, :])

            sm = work.tile([128, N], f16, tag="sm")
            nc.vector.affine_then_add(
                sm[:], adj_t[:], fdb_sb[:], NEG_BIG, fs_cols[:, t:t + 1]
            )
            lr = work.tile([128, N], f16, tag="lr")
            nc.vector.scalar_tensor_tensor(
                lr[:], sm[:], ALPHA, sm[:], A.mult, A.max
            )

            smT_ps = ps_smT.tile([128, N], f16, tag="smT")
            for u in range(TB):
                nc.tensor.transpose(
                    smT_ps[:, u * 128:(u + 1) * 128],
                    lr[:, u * 128:(u + 1) * 128],
                    idf16[:],
                )
            eT = work.tile([128, N], f16, tag="eT")
            nc.scalar.activation(eT[:], smT_ps[:], AF.Exp)

            out_ps = ps_out.tile([128, 130], f32, tag="out")
            for u in range(TB):
                nc.tensor.matmul(
                    out_ps[:],
                    eT[:, u * 128:(u + 1) * 128],
                    nE_v[:, u, :],
                    start=(u == 0),
                    stop=(u == TB - 1),
                )
            rcp = outp.tile([128, 1], f32, tag="rcp")
            nc.vector.reciprocal(rcp[:], out_ps[:, 128:129])
            osb = outp.tile([128, F], f32, tag="osb")
            nc.vector.tensor_scalar(
                osb[:], out_ps[:, 0:F], rcp[:], None, A.mult
            )
            nc.sync.dma_start(out_d[t * 128:(t + 1) * 128, :], osb[:])

    nc.compile()
    return nc


def make_in_maps(inputs, adjs, W_w, W_b, a_w, a_b):
    xT = np.ascontiguousarray(
        np.asarray(inputs, dtype=np.float32).transpose(0, 2, 1))
    adjs = np.ascontiguousarray(adjs, dtype=np.int32)
    wwT = np.ascontiguousarray(np.asarray(W_w, dtype=np.float32).T)
    wb = np.ascontiguousarray(W_b, dtype=np.float32).reshape(F, 1)
    a12 = np.stack(
        [np.asarray(a_w, dtype=np.float32)[0, :F],
         np.asarray(a_w, dtype=np.float32)[0, F:]], axis=1
    ).copy()  # [F, 2]
    ab2 = np.array([[np.asarray(a_b, dtype=np.float32).reshape(())], [0.0]],
                   dtype=np.float32)
    idf16 = np.eye(128, dtype=np.float16)
    return [
        {
            "xT": xT[b],
            "idf16": idf16,
            "adj": adjs[b],
            "WwT": wwT,
            "Wb": wb,
            "a12": a12,
            "ab2": ab2,
        }
        for b in range(BS)
    ]


def kernel(inputs, adjs, W_w, W_b, a_w, a_b):
    from concourse.bass_utils import run_bass_kernel_spmd

    if "nc" not in _cache:
        _cache["nc"] = _build()
    nc = _cache["nc"]

    in_maps = make_in_maps(inputs, adjs, W_w, W_b, a_w, a_b)
    try:
        res = run_bass_kernel_spmd(nc, in_maps, core_ids=list(range(BS)))
    except Exception:
        # transient NRT_EXEC_UNIT_UNRECOVERABLE etc. — retry once
        res = run_bass_kernel_spmd(nc, in_maps, core_ids=list(range(BS)))
    out = np.stack([res.results[b]["out"] for b in range(BS)], axis=0)
    return out.astype(np.float32)

